# revision 1
# baseline (speedup 1.0000x reference)
"""LongNet dilated-attention transformer block on 8 Trainium2 NeuronCores.

Key structural insight: all three branches use rate=2 with even segment
sizes (512/1024/2048), so every branch operates on exactly the same
subsampled token set x[:, ::2, :] (even positions). The branches differ
only in the block size of their block-diagonal attention over those 4096
tokens per batch: m = seg/rate in {256, 512, 1024}.

Sharding: flatten (B=4, 4096 even tokens) -> 16384 tokens -> 8 shards of
2048 tokens (half a batch each). 2048 is a multiple of the largest block
(1024), so every attention block lives entirely within one shard ->
embarrassingly data-parallel, no collectives. Weights are broadcast.

Each core computes, for its (2048, 1024) token shard:
    sum over branches of:
      qkv = x @ W_qkv^T + b ; block-diag attention (block m_i) ; @ W_out^T + b_out
and the host reassembles (4, 4096, 1024).
"""

import numpy as np
from functools import partial

import jax
import jax.numpy as jnp

D = 1024
H = 16
HD = D // H
RATES = (2, 2, 2)
SEGS = (512, 1024, 2048)
BLOCKS = tuple(s // r for s, r in zip(SEGS, RATES))  # (256, 512, 1024)
B, S = 4, 8192
SCALE = 1.0 / np.float32(np.sqrt(HD))
N_CORES = 8
T = (B * S // 2) // N_CORES  # 2048 tokens per core


def _branch(x, m, qkv_w, qkv_b, out_w, out_b):
    """x: (T, D) shard; block-diagonal attention with block size m."""
    t, d = x.shape
    n = t // m
    qkv = x @ qkv_w.T + qkv_b                      # (T, 3D)
    q, k, v = jnp.split(qkv, 3, axis=-1)
    q = q.reshape(n, m, H, HD)
    k = k.reshape(n, m, H, HD)
    v = v.reshape(n, m, H, HD)
    s = jnp.einsum('nqhd,nkhd->nhqk', q, k) * SCALE
    a = jax.nn.softmax(s, axis=-1)
    o = jnp.einsum('nhqk,nkhd->nqhd', a, v).reshape(t, d)
    return o @ out_w.T + out_b


@partial(jax.pmap, in_axes=(0,) + (None,) * 12)
def _shard_fn(x, qw0, qb0, ow0, ob0, qw1, qb1, ow1, ob1, qw2, qb2, ow2, ob2):
    params = ((qw0, qb0, ow0, ob0), (qw1, qb1, ow1, ob1), (qw2, qb2, ow2, ob2))
    out = None
    for m, (qw, qb, ow, ob) in zip(BLOCKS, params):
        y = _branch(x, m, qw, qb, ow, ob)
        out = y if out is None else out + y
    return out


def kernel(x, qkv_w0, qkv_b0, out_w0, out_b0,
           qkv_w1, qkv_b1, out_w1, out_b1,
           qkv_w2, qkv_b2, out_w2, out_b2):
    x = np.asarray(x)
    # even tokens; (B, 4096, D) -> (8, 2048, D)
    xe = np.ascontiguousarray(x[:, ::2, :]).reshape(N_CORES, T, D)
    args = [np.asarray(a) for a in
            (qkv_w0, qkv_b0, out_w0, out_b0,
             qkv_w1, qkv_b1, out_w1, out_b1,
             qkv_w2, qkv_b2, out_w2, out_b2)]
    y = _shard_fn(jnp.asarray(xe), *[jnp.asarray(a) for a in args])
    y = np.asarray(jax.device_get(y))              # (8, 2048, D)
    return y.reshape(B, S // 2, D)
